# revision 4
# baseline (speedup 1.0000x reference)
"""Trainium2 Bass kernel for nn_ConcatNet_5781025980901 (GNN message passing).

8-core SPMD strategy (edge/graph parallelism, refactored for TRN2):
  * Sort edges by destination on the host; partition NODES into 8 contiguous
    ranges with ~equal incoming-edge counts.  Each core owns its node range
    and ALL edges pointing into it, so per-node aggregation is fully local
    (no all-reduce of sums needed).
  * Algebraic refactor of the per-edge message MLP:
        u = [x_i, x_j, e] @ Wm1 + bm1
          = a[dst] + b[src] + (relu(ea@We1+be1) @ (We2@Wm1_e)) + const
    with per-node tables a = t@(Wn2@Wm1_a), b = t@(Wn2@Wm1_b),
    t = relu(x@Wn1+bn1).  This removes the node-MLP from the edge loop.
  * The second message layer commutes with the segment sum:
        mean(relu(u) @ Wm2 + bm2) = (wmean relu(u)) @ Wm2 + gate*bm2
    so Wm2 is applied per-node AFTER aggregation.
  * Segment mean is computed on the TensorEngine: for each 128-node block,
    one-hot selection matrices S'[e, n] = (dst_rel==n) * inv_deg[dst] are
    built with a single fused vector op and matmul-accumulated into PSUM.
  * b tables are exchanged with an on-chip AllGather each layer; x stays
    sharded (transposed) the whole time.

kernel(**inputs) takes the full unsharded inputs and returns the full
[50000, 64] output.  Self-contained: only needs the concourse runtime
at /opt/trn_rl_repo.
"""
import sys

if "/opt/trn_rl_repo" not in sys.path:
    sys.path.insert(0, "/opt/trn_rl_repo")

from contextlib import ExitStack

import numpy as np

from concourse import bacc, bass, mybir, tile
from concourse.bass import IndirectOffsetOnAxis
from concourse.bass_utils import run_bass_kernel_spmd
from concourse.masks import make_identity

P = 128
NCORES = 8
L = 3
EPS = 1e-5
F32 = mybir.dt.float32
I32 = mybir.dt.int32
AF = mybir.ActivationFunctionType
OP = mybir.AluOpType


# ---------------------------------------------------------------- host prep

def _host_prep(inputs):
    src = np.asarray(inputs["edge_index"][0])
    dst = np.asarray(inputs["edge_index"][1])
    x = np.asarray(inputs["x"], dtype=np.float32)
    ea = np.asarray(inputs["edge_attr"], dtype=np.float32)
    N = x.shape[0]
    E = src.shape[0]

    deg = np.bincount(dst, minlength=N).astype(np.float32)
    inv_denom = (1.0 / np.maximum(deg, 1.0)).astype(np.float32)
    gate = (deg > 0).astype(np.float32)

    perm = np.argsort(dst, kind="stable")
    s_dst = dst[perm]
    s_src = src[perm]
    s_ea = ea[perm]

    # node ranges with ~equal edge counts
    cum = np.cumsum(deg)
    bounds = [0] + [int(np.searchsorted(cum, E * c / NCORES))
                    for c in range(1, NCORES)] + [N]
    n_lo = np.array(bounds[:-1])
    n_hi = np.array(bounds[1:])
    Nc = n_hi - n_lo
    NB = (int(Nc.max()) + P - 1) // P
    NS = NB * P

    e_lo = np.searchsorted(s_dst, n_lo)
    e_hi = np.searchsorted(s_dst, n_hi)

    core_of = np.repeat(np.arange(NCORES), Nc)
    src_padded = (core_of[s_src] * NS + (s_src - n_lo[core_of[s_src]])).astype(np.int64)

    # per-core edge tiling: block = 128 consecutive node slots; each block
    # gets exactly S_max subtiles of 128 edges (padded)
    per_core_meta = []
    S_max = 1
    for c in range(NCORES):
        d_loc = s_dst[e_lo[c]:e_hi[c]] - n_lo[c]
        blk = d_loc >> 7
        blk_starts = np.searchsorted(blk, np.arange(NB))
        r = np.arange(len(d_loc)) - blk_starts[blk]
        per_core_meta.append((d_loc, blk, r))
        if len(r):
            S_max = max(S_max, int(r.max()) // P + 1)
    T = NB * S_max

    per_core = []
    for c in range(NCORES):
        d_loc, blk, r = per_core_meta[c]
        sl = slice(e_lo[c], e_hi[c])
        pos = (blk * S_max + (r >> 7)) * P + (r & 127)

        ea_t = np.zeros((64, T * P), np.float32)
        ea_t[:, pos] = s_ea[sl].T
        dst_rel = np.full(T * P, -1.0, np.float32)
        dst_rel[pos] = (d_loc & 127).astype(np.float32)
        w_e = np.zeros(T * P, np.float32)
        w_e[pos] = inv_denom[s_dst[sl]]
        a_idx = np.zeros(T * P, np.int32)
        a_idx[pos] = d_loc.astype(np.int32)
        b_idx = np.zeros(T * P, np.int32)
        b_idx[pos] = src_padded[sl].astype(np.int32)

        x_t = np.zeros((P, NS), np.float32)
        x_t[:, :Nc[c]] = x[n_lo[c]:n_hi[c]].T
        gate_c = np.zeros((1, NS), np.float32)
        gate_c[0, :Nc[c]] = gate[n_lo[c]:n_hi[c]]

        per_core.append({
            "ea_t": ea_t,
            "dst_rel": dst_rel.reshape(T, P).T.copy(),
            "w_e": w_e.reshape(T, P).T.copy(),
            "a_idx": a_idx.reshape(T, P).T.copy(),
            "b_idx": b_idx.reshape(T, P).T.copy(),
            "x_t": x_t,
            "gate": gate_c,
        })

    # folded weights, packed in device-friendly layouts
    Wn1 = np.asarray(inputs["Wn1"], np.float32)
    bn1 = np.asarray(inputs["bn1"], np.float32)
    Wn2 = np.asarray(inputs["Wn2"], np.float32)
    bn2 = np.asarray(inputs["bn2"], np.float32)
    We1 = np.asarray(inputs["We1"], np.float32)
    be1 = np.asarray(inputs["be1"], np.float32)
    We2 = np.asarray(inputs["We2"], np.float32)
    be2 = np.asarray(inputs["be2"], np.float32)
    Wm1 = np.asarray(inputs["Wm1"], np.float32)
    bm1 = np.asarray(inputs["bm1"], np.float32)
    lnw = np.asarray(inputs["lnw"], np.float32)
    lnb = np.asarray(inputs["lnb"], np.float32)

    wna = np.zeros((L, 128, 256), np.float32)
    wnb = np.zeros((L, 128, 256), np.float32)
    wfold = np.zeros((L, 128, 128), np.float32)
    biasu = np.zeros((L, 1, 128), np.float32)
    bn1p = np.zeros((L, 128, 2), np.float32)
    be1p = np.zeros((L, 128, 1), np.float32)
    for l in range(L):
        Wm1a, Wm1b, Wm1e = Wm1[l][:128], Wm1[l][128:256], Wm1[l][256:320]
        WnA = Wn2[l] @ Wm1a
        WnB = Wn2[l] @ Wm1b
        wna[l] = np.concatenate([WnA[:128], WnA[128:]], axis=1)
        wnb[l] = np.concatenate([WnB[:128], WnB[128:]], axis=1)
        wfold[l] = We2[l] @ Wm1e
        biasu[l, 0] = bn2[l] @ Wm1a + bn2[l] @ Wm1b + be2[l] @ Wm1e + bm1[l]
        bn1p[l, :, 0] = bn1[l][:128]
        bn1p[l, :, 1] = bn1[l][128:]
        be1p[l, :, 0] = be1[l]

    use_ln_affine = not (np.all(lnw == 1.0) and np.all(lnb == 0.0))

    shared = {
        "wn1": Wn1,                      # [L,128,256] lhsT chunks
        "bn1p": bn1p,                    # [L,128,2]
        "wna": wna, "wnb": wnb,          # [L,128,256] rhs chunks
        "biasu": biasu,                  # [L,1,128]
        "we1": We1,                      # [L,64,128] lhsT
        "be1p": be1p,                    # [L,128,1]
        "wfold": wfold,                  # [L,128,128] rhs
        "wm2": np.asarray(inputs["Wm2"], np.float32),   # [L,128,128] rhs
        "bm2p": np.asarray(inputs["bm2"], np.float32)[:, None, :],  # [L,1,128]
        "wf": np.asarray(inputs["Wf"], np.float32),     # [128,64] rhs
        "bfp": np.asarray(inputs["bf"], np.float32)[None, :],       # [1,64]
    }
    if use_ln_affine:
        shared["lnwp"] = lnw[:, None, :]   # [L,1,128]
        shared["lnbp"] = lnb[:, None, :]

    cfg = dict(NB=NB, NS=NS, S_max=S_max, T=T, N=N,
               use_ln_affine=use_ln_affine)
    meta = dict(n_lo=n_lo, n_hi=n_hi, Nc=Nc)
    return per_core, shared, cfg, meta


# ------------------------------------------------------------- device build

def build_program(cfg):
    NB, NS, S_max, T = cfg["NB"], cfg["NS"], cfg["S_max"], cfg["T"]
    affine = cfg["use_ln_affine"]

    nc = bacc.Bacc("TRN2", target_bir_lowering=False, debug=False,
                   enable_asserts=False, num_devices=NCORES)

    # I/O
    d_x = nc.dram_tensor("x_t", [P, NS], F32, kind="ExternalInput")
    d_ea = nc.dram_tensor("ea_t", [64, T * P], F32, kind="ExternalInput")
    d_dr = nc.dram_tensor("dst_rel", [P, T], F32, kind="ExternalInput")
    d_we = nc.dram_tensor("w_e", [P, T], F32, kind="ExternalInput")
    d_ai = nc.dram_tensor("a_idx", [P, T], I32, kind="ExternalInput")
    d_bi = nc.dram_tensor("b_idx", [P, T], I32, kind="ExternalInput")
    d_gate = nc.dram_tensor("gate", [1, NS], F32, kind="ExternalInput")
    d_wn1 = nc.dram_tensor("wn1", [L, P, 256], F32, kind="ExternalInput")
    d_bn1 = nc.dram_tensor("bn1p", [L, P, 2], F32, kind="ExternalInput")
    d_wna = nc.dram_tensor("wna", [L, P, 256], F32, kind="ExternalInput")
    d_wnb = nc.dram_tensor("wnb", [L, P, 256], F32, kind="ExternalInput")
    d_bu = nc.dram_tensor("biasu", [L, 1, P], F32, kind="ExternalInput")
    d_we1 = nc.dram_tensor("we1", [L, 64, P], F32, kind="ExternalInput")
    d_be1 = nc.dram_tensor("be1p", [L, P, 1], F32, kind="ExternalInput")
    d_wfold = nc.dram_tensor("wfold", [L, P, P], F32, kind="ExternalInput")
    d_wm2 = nc.dram_tensor("wm2", [L, P, P], F32, kind="ExternalInput")
    d_bm2 = nc.dram_tensor("bm2p", [L, 1, P], F32, kind="ExternalInput")
    d_wf = nc.dram_tensor("wf", [P, 64], F32, kind="ExternalInput")
    d_bf = nc.dram_tensor("bfp", [1, 64], F32, kind="ExternalInput")
    if affine:
        d_lnw = nc.dram_tensor("lnwp", [L, 1, P], F32, kind="ExternalInput")
        d_lnb = nc.dram_tensor("lnbp", [L, 1, P], F32, kind="ExternalInput")
    d_out = nc.dram_tensor("out", [NS, 64], F32, kind="ExternalOutput")

    # internal DRAM scratch
    d_xb = [nc.dram_tensor(f"xbuf{i}", [P, NS], F32) for i in range(2)]
    d_aloc = nc.dram_tensor("a_loc", [NS, P], F32)
    d_bloc = nc.dram_tensor("b_loc", [NS, P], F32)
    d_bfull = nc.dram_tensor("b_full", [NCORES * NS, P], F32)

    with tile.TileContext(nc) as tc, ExitStack() as ctx:
        cpool = ctx.enter_context(tc.tile_pool(name="consts", bufs=1))
        wpool = ctx.enter_context(tc.tile_pool(name="wts", bufs=1))
        work = ctx.enter_context(tc.tile_pool(name="work", bufs=3))
        blkp = ctx.enter_context(tc.tile_pool(name="blk", bufs=2))
        pA = ctx.enter_context(tc.tile_pool(name="pA", bufs=2, space="PSUM"))
        pB = ctx.enter_context(tc.tile_pool(name="pB", bufs=2, space="PSUM"))
        pP = ctx.enter_context(tc.tile_pool(name="pP", bufs=2, space="PSUM"))
        pQ = ctx.enter_context(tc.tile_pool(name="pQ", bufs=2, space="PSUM"))

        # constants
        ident = cpool.tile([P, P], F32)
        make_identity(nc, ident)
        iota = cpool.tile([P, P], I32)
        nc.gpsimd.iota(iota, pattern=[[1, P]], base=0, channel_multiplier=0)
        iota_f = cpool.tile([P, P], F32)
        nc.vector.tensor_copy(out=iota_f, in_=iota)
        ones_row = cpool.tile([1, P], F32)
        nc.vector.memset(ones_row, 1.0)
        eps_col = cpool.tile([P, 1], F32)
        nc.vector.memset(eps_col, EPS)

        gate_sb = cpool.tile([1, NS], F32)
        nc.sync.dma_start(out=gate_sb, in_=d_gate[:, :])
        dr_sb = cpool.tile([P, T], F32)
        nc.sync.dma_start(out=dr_sb, in_=d_dr[:, :])
        we_sb = cpool.tile([P, T], F32)
        nc.sync.dma_start(out=we_sb, in_=d_we[:, :])
        ai_sb = cpool.tile([P, T], I32)
        nc.sync.dma_start(out=ai_sb, in_=d_ai[:, :])
        bi_sb = cpool.tile([P, T], I32)
        nc.sync.dma_start(out=bi_sb, in_=d_bi[:, :])
        wf_sb = cpool.tile([P, 64], F32)
        nc.sync.dma_start(out=wf_sb, in_=d_wf[:, :])
        bf_sb = cpool.tile([1, 64], F32)
        nc.sync.dma_start(out=bf_sb, in_=d_bf[:, :])

        for l in range(L):
            x_src = d_x if l == 0 else d_xb[(l + 1) % 2]
            x_dstb = d_xb[l % 2]

            # per-layer weights
            wn1_sb = wpool.tile([P, 256], F32, tag="wn1")
            nc.sync.dma_start(out=wn1_sb, in_=d_wn1[l])
            bn1_sb = wpool.tile([P, 2], F32, tag="bn1")
            nc.sync.dma_start(out=bn1_sb, in_=d_bn1[l])
            wna_sb = wpool.tile([P, 256], F32, tag="wna")
            nc.sync.dma_start(out=wna_sb, in_=d_wna[l])
            wnb_sb = wpool.tile([P, 256], F32, tag="wnb")
            nc.sync.dma_start(out=wnb_sb, in_=d_wnb[l])
            bu_sb = wpool.tile([1, P], F32, tag="bu")
            nc.sync.dma_start(out=bu_sb, in_=d_bu[l])
            we1_sb = wpool.tile([64, P], F32, tag="we1")
            nc.sync.dma_start(out=we1_sb, in_=d_we1[l])
            be1_sb = wpool.tile([P, 1], F32, tag="be1")
            nc.sync.dma_start(out=be1_sb, in_=d_be1[l])
            wfold_sb = wpool.tile([P, P], F32, tag="wfold")
            nc.sync.dma_start(out=wfold_sb, in_=d_wfold[l])
            wm2_sb = wpool.tile([P, P], F32, tag="wm2")
            nc.sync.dma_start(out=wm2_sb, in_=d_wm2[l])
            bm2_sb = wpool.tile([1, P], F32, tag="bm2")
            nc.sync.dma_start(out=bm2_sb, in_=d_bm2[l])
            if affine:
                lnw_sb = wpool.tile([P, P], F32, tag="lnw")
                nc.sync.dma_start(out=lnw_sb, in_=d_lnw[l].to_broadcast([P, P]))
                lnb_sb = wpool.tile([P, P], F32, tag="lnb")
                nc.sync.dma_start(out=lnb_sb, in_=d_lnb[l].to_broadcast([P, P]))

            # ---- node phase: a_loc, b_loc tables ----
            for b in range(NB):
                ns = slice(b * P, (b + 1) * P)
                xt_in = work.tile([P, P], F32, tag="xt_in")
                nc.sync.dma_start(out=xt_in, in_=x_src[:, ns])
                t0_ps = pA.tile([P, P], F32, tag="pA")
                nc.tensor.matmul(out=t0_ps, lhsT=wn1_sb[:, 0:P], rhs=xt_in,
                                 start=True, stop=True)
                t1_ps = pB.tile([P, P], F32, tag="pB")
                nc.tensor.matmul(out=t1_ps, lhsT=wn1_sb[:, P:2 * P], rhs=xt_in,
                                 start=True, stop=True)
                t0r = work.tile([P, P], F32, tag="t0r")
                nc.scalar.activation(out=t0r, in_=t0_ps, func=AF.Relu,
                                     bias=bn1_sb[:, 0:1])
                t1r = work.tile([P, P], F32, tag="t1r")
                nc.scalar.activation(out=t1r, in_=t1_ps, func=AF.Relu,
                                     bias=bn1_sb[:, 1:2])
                a_ps = pP.tile([P, P], F32, tag="pP")
                nc.tensor.matmul(out=a_ps, lhsT=t0r, rhs=wna_sb[:, 0:P],
                                 start=True, stop=False)
                nc.tensor.matmul(out=a_ps, lhsT=t1r, rhs=wna_sb[:, P:2 * P],
                                 start=False, stop=False)
                nc.tensor.matmul(out=a_ps, lhsT=ones_row, rhs=bu_sb,
                                 start=False, stop=True)
                a_sb = work.tile([P, P], F32, tag="a_sb")
                nc.vector.tensor_copy(out=a_sb, in_=a_ps)
                nc.sync.dma_start(out=d_aloc[ns, :], in_=a_sb)
                b_ps = pQ.tile([P, P], F32, tag="pQ")
                nc.tensor.matmul(out=b_ps, lhsT=t0r, rhs=wnb_sb[:, 0:P],
                                 start=True, stop=False)
                nc.tensor.matmul(out=b_ps, lhsT=t1r, rhs=wnb_sb[:, P:2 * P],
                                 start=False, stop=True)
                b_sb = work.tile([P, P], F32, tag="b_sb")
                nc.scalar.activation(out=b_sb, in_=b_ps, func=AF.Copy)
                nc.sync.dma_start(out=d_bloc[ns, :], in_=b_sb)

            nc.gpsimd.collective_compute(
                "AllGather", OP.bypass,
                replica_groups=[list(range(NCORES))],
                ins=[d_bloc[:, :].opt()],
                outs=[d_bfull[:, :].opt()],
            )

            # ---- edge phase ----
            for b in range(NB):
                ns = slice(b * P, (b + 1) * P)
                ea_blk = work.tile([64, S_max * P], F32, tag="ea")
                nc.sync.dma_start(
                    out=ea_blk, in_=d_ea[:, b * S_max * P:(b + 1) * S_max * P])
                P_ps = pP.tile([P, P], F32, tag="pP")
                for s in range(S_max):
                    t = b * S_max + s
                    h1_ps = pA.tile([P, P], F32, tag="pA")
                    nc.tensor.matmul(out=h1_ps, lhsT=we1_sb,
                                     rhs=ea_blk[:, s * P:(s + 1) * P],
                                     start=True, stop=True)
                    h1r = work.tile([P, P], F32, tag="h1r")
                    nc.scalar.activation(out=h1r, in_=h1_ps, func=AF.Relu,
                                         bias=be1_sb[:, 0:1])
                    c_ps = pB.tile([P, P], F32, tag="pB")
                    nc.tensor.matmul(out=c_ps, lhsT=h1r, rhs=wfold_sb,
                                     start=True, stop=True)
                    a_g = work.tile([P, P], F32, tag="a_g")
                    nc.gpsimd.indirect_dma_start(
                        out=a_g, out_offset=None, in_=d_aloc[:, :],
                        in_offset=IndirectOffsetOnAxis(ap=ai_sb[:, t:t + 1], axis=0))
                    b_g = work.tile([P, P], F32, tag="b_g")
                    nc.gpsimd.indirect_dma_start(
                        out=b_g, out_offset=None, in_=d_bfull[:, :],
                        in_offset=IndirectOffsetOnAxis(ap=bi_sb[:, t:t + 1], axis=0))
                    ab = work.tile([P, P], F32, tag="ab")
                    nc.gpsimd.tensor_tensor(out=ab, in0=a_g, in1=b_g, op=OP.add)
                    u = work.tile([P, P], F32, tag="u")
                    nc.vector.tensor_tensor(out=u, in0=c_ps, in1=ab, op=OP.add)
                    ru = work.tile([P, P], F32, tag="ru")
                    nc.scalar.activation(out=ru, in_=u, func=AF.Relu)
                    s_sb = work.tile([P, P], F32, tag="s_sb")
                    nc.vector.tensor_scalar(
                        out=s_sb, in0=iota_f, scalar1=dr_sb[:, t:t + 1],
                        scalar2=we_sb[:, t:t + 1],
                        op0=OP.is_equal, op1=OP.mult)
                    nc.tensor.matmul(out=P_ps, lhsT=s_sb, rhs=ru,
                                     start=(s == 0), stop=(s == S_max - 1))

                # block tail: agg = P̄@Wm2 + gate*bm2 ; LayerNorm ; relu
                pb_sb = blkp.tile([P, P], F32, tag="pb_sb")
                nc.vector.tensor_copy(out=pb_sb, in_=P_ps)
                pt_ps = pQ.tile([P, P], F32, tag="pQ")
                nc.tensor.transpose(out=pt_ps, in_=pb_sb, identity=ident)
                pt_sb = blkp.tile([P, P], F32, tag="pt_sb")
                nc.scalar.activation(out=pt_sb, in_=pt_ps, func=AF.Copy)
                agg_ps = pQ.tile([P, P], F32, tag="pQ")
                nc.tensor.matmul(out=agg_ps, lhsT=pt_sb, rhs=wm2_sb,
                                 start=True, stop=False)
                nc.tensor.matmul(out=agg_ps, lhsT=gate_sb[:, ns], rhs=bm2_sb,
                                 start=False, stop=True)
                stats = blkp.tile([P, 6], F32, tag="stats")
                nc.vector.bn_stats(out=stats, in_=agg_ps)
                mv = blkp.tile([P, 2], F32, tag="mv")
                nc.vector.bn_aggr(out=mv, in_=stats)
                nc.scalar.activation(out=mv[:, 1:2], in_=mv[:, 1:2],
                                     func=AF.Sqrt, bias=eps_col)
                nc.vector.reciprocal(out=mv[:, 1:2], in_=mv[:, 1:2])
                z = blkp.tile([P, P], F32, tag="z")
                nc.vector.tensor_scalar(
                    out=z, in0=agg_ps, scalar1=mv[:, 0:1], scalar2=mv[:, 1:2],
                    op0=OP.subtract, op1=OP.mult)
                if affine:
                    nc.vector.tensor_tensor(out=z, in0=z, in1=lnw_sb, op=OP.mult)
                    nc.vector.tensor_tensor(out=z, in0=z, in1=lnb_sb, op=OP.add)
                xn = blkp.tile([P, P], F32, tag="xn")
                nc.scalar.activation(out=xn, in_=z, func=AF.Relu)
                xt_ps = pQ.tile([P, P], F32, tag="pQ")
                nc.tensor.transpose(out=xt_ps, in_=xn, identity=ident)
                xt_sb = blkp.tile([P, P], F32, tag="xt_sb")
                nc.vector.tensor_copy(out=xt_sb, in_=xt_ps)
                if l < L - 1:
                    nc.sync.dma_start(out=x_dstb[:, ns], in_=xt_sb)
                else:
                    out_ps = pQ.tile([P, 64], F32, tag="pQ")
                    nc.tensor.matmul(out=out_ps, lhsT=xt_sb, rhs=wf_sb,
                                     start=True, stop=False)
                    nc.tensor.matmul(out=out_ps, lhsT=ones_row, rhs=bf_sb,
                                     start=False, stop=True)
                    ob = blkp.tile([P, 64], F32, tag="ob")
                    nc.vector.tensor_copy(out=ob, in_=out_ps)
                    nc.sync.dma_start(out=d_out[ns, :], in_=ob)

    nc.compile()
    return nc


# ---------------------------------------------------------------- entry

_CACHE = {}


def _get_program(cfg):
    key = (cfg["NB"], cfg["NS"], cfg["S_max"], cfg["T"], cfg["use_ln_affine"])
    if key not in _CACHE:
        _CACHE[key] = build_program(cfg)
    return _CACHE[key]


def run(inputs, trace=False):
    per_core, shared, cfg, meta = _host_prep(inputs)
    nc = _get_program(cfg)
    in_maps = [{**pc, **shared} for pc in per_core]
    res = run_bass_kernel_spmd(nc, in_maps, list(range(NCORES)), trace=trace)
    N = cfg["N"]
    full = np.empty((N, 64), np.float32)
    for c in range(NCORES):
        full[meta["n_lo"][c]:meta["n_hi"][c]] = \
            res.results[c]["out"][:meta["Nc"][c]]
    return full, res


def kernel(**inputs):
    full, _ = run(inputs)
    return full
